# revision 1
# baseline (speedup 1.0000x reference)
"""Trainium2 Bass kernel for nn_ConditionalSplineSQ2D.

Math:
  out[b] = sum_{g,h,c} coeffs[g,h,c] * p[b,g,h,ii_c] * p[b,g,h,jj_c]
         = sum_{cells} p_cell^T S_cell p_cell            (S_cell symmetric 8x8)
         = sum_{cells} sum_k lam[cell,k] * (V[cell]^T p_cell)_k^2

Host precomputes the eigendecomposition of the 961 8x8 matrices; the device
kernel per 16-cell group does:
  mm1 (PE, 4x col-tiled): T = Wblk^T @ P    (block-diag stationary, fp16)
  sq  (ACT or DVE):       Q = T*T           (PSUM -> SBUF fp16, 3 groups/op)
  mm2 (PE, 4x col-tiled): acc[32j,:] += lam_g^T @ Q_g  (4 groups concurrent)

Sharding: pure data parallel over batch (512 per core x 8 cores); the
4 partial accumulator rows per core are summed on host.
"""

import numpy as np

B, G, P = 4096, 31, 8
NCORES = 8
NB = B // NCORES  # 512 batches per core
CELLS = G * G  # 961
GROUP_CELLS = 16
NGROUPS = -(-CELLS // GROUP_CELLS)  # 61
CELLS_PAD = NGROUPS * GROUP_CELLS  # 976
PARTS = 128
TG = 3   # groups per PSUM tile (3 banks)
NT = -(-NGROUPS // TG)  # 21 psum tiles
ACT_COLS = 960  # per-tile square columns done by ScalarE; rest on VectorE
# DMA chunk sizes in groups: small first chunks so mm1 starts early
_CHUNKS = [2, 4] + [8] * ((NGROUPS - 6 + 7) // 8)
_CHUNKS[-1] -= sum(_CHUNKS) - NGROUPS
assert sum(_CHUNKS) == NGROUPS and all(c > 0 for c in _CHUNKS)

_nc_cache = {}


def _build_nc():
    import concourse.mybir as mybir
    import concourse.tile as tile
    from concourse import bacc

    nc = bacc.Bacc()
    pt_d = nc.dram_tensor(
        "pt", [PARTS, NGROUPS * NB], mybir.dt.float16, kind="ExternalInput"
    )
    w_d = nc.dram_tensor(
        "wblk", [PARTS, NGROUPS * PARTS], mybir.dt.float16, kind="ExternalInput"
    )
    lam_d = nc.dram_tensor(
        "lamt", [PARTS, NGROUPS], mybir.dt.float16, kind="ExternalInput"
    )
    out_d = nc.dram_tensor("out", [4, NB], mybir.dt.float32, kind="ExternalOutput")

    with tile.TileContext(nc) as tc:
        with (
            tc.tile_pool(name="const", bufs=1) as cpool,
            tc.tile_pool(name="ptp", bufs=4) as ppool,
            tc.tile_pool(name="qp", bufs=3) as qpool,
            tc.tile_pool(name="psp", bufs=2, space="PSUM") as pspool,
            tc.tile_pool(name="accp", bufs=1, space="PSUM") as apool,
        ):
            w_sb = cpool.tile([PARTS, NGROUPS * PARTS], mybir.dt.float16)
            lam_sb = cpool.tile([PARTS, NGROUPS], mybir.dt.float16)
            acc = apool.tile([PARTS, NB], mybir.dt.float32)

            # stream param + weights; chunk->groups map, small chunks first
            ptiles = []       # (ptile, first_group)
            group_chunk = {}  # group -> (chunk_idx, offset_in_chunk)
            g0 = 0
            for ci, ch in enumerate(_CHUNKS):
                nc.sync.dma_start(
                    out=w_sb[:, g0 * PARTS : (g0 + ch) * PARTS],
                    in_=w_d[:, g0 * PARTS : (g0 + ch) * PARTS],
                )
                ptile = ppool.tile([PARTS, max(_CHUNKS) * NB], mybir.dt.float16,
                                   tag="ptile")
                nc.sync.dma_start(
                    out=ptile[:, : ch * NB],
                    in_=pt_d[:, g0 * NB : (g0 + ch) * NB],
                )
                ptiles.append(ptile)
                for k in range(ch):
                    group_chunk[g0 + k] = (ci, k)
                if ci == 0:
                    nc.sync.dma_start(out=lam_sb[:, :], in_=lam_d[:, :])
                g0 += ch

            q_slices = {}  # group -> (q_tile, slot)
            n_rounds = -(-NGROUPS // 4)  # 16 mm2 rounds of up to 4 groups
            rounds_emitted = 0
            last_round_of_pos = {}  # col pos j -> last round index using it
            for r in range(n_rounds):
                for j in range(4):
                    if r * 4 + j < NGROUPS:
                        last_round_of_pos[j] = r

            def emit_mm2_rounds(limit_group):
                """Emit mm2 rounds whose groups are all squared (< limit)."""
                nonlocal rounds_emitted
                while rounds_emitted < n_rounds:
                    r = rounds_emitted
                    hi = min(r * 4 + 4, NGROUPS)
                    if hi > limit_group:
                        return
                    for j in range(4):
                        g = r * 4 + j
                        if g >= NGROUPS:
                            break
                        qt, slot = q_slices.pop(g)
                        nc.tensor.matmul(
                            acc[32 * j : 32 * j + 1, :],
                            lam_sb[:, g : g + 1],
                            qt[:, slot * NB : (slot + 1) * NB],
                            start=(r == 0),
                            stop=(r == last_round_of_pos[j]),
                            tile_position=(0, 32 * j),
                        )
                    rounds_emitted += 1

            for t in range(NT):
                tg0 = t * TG
                ng = min(TG, NGROUPS - tg0)
                psT = pspool.tile([PARTS, TG * NB], mybir.dt.float32, tag="psT")
                for gg in range(ng):
                    g = tg0 + gg
                    ci, off = group_chunk[g]
                    # mm1: 4 concurrent 128x32 column tiles
                    for j in range(4):
                        nc.tensor.matmul(
                            psT[32 * j : 32 * j + 32, gg * NB : (gg + 1) * NB],
                            w_sb[:, g * PARTS + 32 * j : g * PARTS + 32 * j + 32],
                            ptiles[ci][:, off * NB : (off + 1) * NB],
                            start=True,
                            stop=True,
                            tile_position=(0, 32 * j),
                        )
                # square split across BOTH engines by columns so every tile
                # costs ~1us on each engine (uniform pipeline cadence)
                q = qpool.tile([PARTS, TG * NB], mybir.dt.float16, tag="q")
                ncols = ng * NB
                a_cols = (ncols * ACT_COLS) // (TG * NB)
                nc.scalar.square(q[:, :a_cols], psT[:, :a_cols])
                if a_cols < ncols:
                    qc = qpool.tile([PARTS, TG * NB], mybir.dt.float16, tag="qc")
                    nc.vector.tensor_copy(
                        qc[:, a_cols:ncols], psT[:, a_cols:ncols]
                    )
                    nc.vector.tensor_mul(
                        q[:, a_cols:ncols], qc[:, a_cols:ncols], qc[:, a_cols:ncols]
                    )
                for gg in range(ng):
                    q_slices[tg0 + gg] = (q, gg)
                # weight-reduce every fully-squared round of 4 groups, but
                # stay one tile behind so PE never waits on a fresh square
                emit_mm2_rounds(tg0)
            emit_mm2_rounds(NGROUPS)

            # evacuate the 4 accumulator rows (partition-aligned copies)
            out_sb = cpool.tile([PARTS, NB], mybir.dt.float32)
            for j in range(4):
                nc.vector.tensor_copy(
                    out_sb[32 * j : 32 * j + 1, :], acc[32 * j : 32 * j + 1, :]
                )
                nc.sync.dma_start(
                    out=out_d[j : j + 1, :], in_=out_sb[32 * j : 32 * j + 1, :]
                )
    if not nc.is_finalized():
        nc.finalize()
    return nc


def _get_nc():
    if "nc" not in _nc_cache:
        _nc_cache["nc"] = _build_nc()
    return _nc_cache["nc"]


def _host_prep_weights(integral_coeffs):
    """coeffs [G,G,C] -> (wblk [128, NGROUPS*128] fp16, lamt [128, NGROUPS] fp16)."""
    ii, jj = np.triu_indices(P)
    w = integral_coeffs.reshape(CELLS, len(ii)).astype(np.float64)
    S = np.zeros((CELLS, P, P), np.float64)
    # quadratic form: off-diag split in half, diag gets full coeff
    np.add.at(S, (slice(None), ii, jj), 0.5 * w)
    np.add.at(S, (slice(None), jj, ii), 0.5 * w)
    lam, V = np.linalg.eigh(S)  # V columns are eigenvectors

    lam_p = np.zeros((CELLS_PAD, P))
    lam_p[:CELLS] = lam
    V_p = np.zeros((CELLS_PAD, P, P))
    V_p[:CELLS] = V

    # block-diagonal stationary: wb[g, 8t+i, 8t+k] = V[16g+t, i, k]
    Vg = V_p.reshape(NGROUPS, GROUP_CELLS, P, P)
    wb = np.zeros((NGROUPS, GROUP_CELLS, P, GROUP_CELLS, P), np.float32)
    t = np.arange(GROUP_CELLS)
    wb[:, t, :, t, :] = Vg.transpose(1, 0, 2, 3)
    wblk = (
        wb.reshape(NGROUPS, PARTS, PARTS)
        .transpose(1, 0, 2)
        .reshape(PARTS, NGROUPS * PARTS)
        .astype(np.float16)
    )
    lamt = np.ascontiguousarray(
        lam_p.reshape(NGROUPS, PARTS).T.astype(np.float16)
    )
    return np.ascontiguousarray(wblk), lamt


def _host_prep_param(param_tensor):
    """param [B,G,G,P] f32 -> list of per-core [128, NGROUPS*NB] fp16 arrays."""
    flat = param_tensor.reshape(B, CELLS * P)
    out = []
    for c in range(NCORES):
        shard = flat[c * NB : (c + 1) * NB]
        pad = np.zeros((NB, CELLS_PAD * P), np.float32)
        pad[:, : CELLS * P] = shard
        # (b, g, p) -> (p, g, b)
        pt = (
            pad.reshape(NB, NGROUPS, PARTS)
            .transpose(2, 1, 0)
            .reshape(PARTS, NGROUPS * NB)
            .astype(np.float16)
        )
        out.append(np.ascontiguousarray(pt))
    return out


def _run(param_tensor, integral_coeffs, trace=False, **run_kwargs):
    from concourse.bass_utils import run_bass_kernel_spmd

    nc = _get_nc()
    wblk, lamt = _host_prep_weights(np.asarray(integral_coeffs, np.float32))
    pts = _host_prep_param(np.asarray(param_tensor, np.float32))
    in_maps = [{"pt": pts[c], "wblk": wblk, "lamt": lamt} for c in range(NCORES)]
    res = run_bass_kernel_spmd(
        nc, in_maps, core_ids=list(range(NCORES)), trace=trace, **run_kwargs
    )
    out = np.concatenate(
        [res.results[c]["out"].sum(axis=0).reshape(NB) for c in range(NCORES)]
    ).astype(np.float32)
    return out, res


def kernel(param_tensor, integral_coeffs):
    out, _ = _run(param_tensor, integral_coeffs)
    return out



# revision 4
# speedup vs baseline: 1.0722x; 1.0722x over previous
"""Trainium2 Bass kernel for nn_ConditionalSplineSQ2D.

Math:
  out[b] = sum_{g,h,c} coeffs[g,h,c] * p[b,g,h,ii_c] * p[b,g,h,jj_c]
         = sum_{cells} p_cell^T S_cell p_cell            (S_cell symmetric 8x8)

Two equivalent per-group forms, split across engines to balance the
PSUM-drain work:
  EIGEN  (ACT):  T = V^T p (PE), Q = T*T (ScalarE square), out += lam^T Q (PE)
  DIRECT (DVE):  T = S^T p (PE), Z = T .* p (VectorE fused mult), out += 1^T Z (PE)

mm1 uses the 16x 32x32 PE sub-array tiling: each group's block-diag
stationary is 4 independent 32x32 tiles (4 cells each).  Eigen groups
rotate tiles to column-band (r+g)%4 so concurrent groups occupy disjoint
sub-arrays; direct groups stay on the diagonal (c=r) so the PSUM
partition layout matches pt's layout for the elementwise multiply.

Entire per-core input (7.8 MB fp16) is prefetched into SBUF with a few
large DMAs so DMA runs at full rate with no buffer-recycle stalls.

Sharding: pure data parallel over batch (512 per core x 8 cores); the
4 partial accumulator rows per core are summed on host.
"""

import numpy as np

B, G, P = 4096, 31, 8
NCORES = 8
NB = B // NCORES  # 512 batches per core
CELLS = G * G  # 961
GROUP_CELLS = 16
NGROUPS = -(-CELLS // GROUP_CELLS)  # 61
CELLS_PAD = NGROUPS * GROUP_CELLS  # 976
PARTS = 128
WAVE = 3  # groups per wave (one PSUM bank per group, 3-bank wave tiles)
NWAVES = -(-NGROUPS // WAVE)  # 21 (last wave has 1 group)

# DMA chunking (in groups): small first chunks so compute starts early
PT_CHUNKS = [6, 8, 10, 12, 12, 13]
assert sum(PT_CHUNKS) == NGROUPS
WT_CHUNKS = [12, NGROUPS - 12]


def _wave_pattern(w):
    """Group forms for wave w, E(igen/ACT) first then D(irect/DVE)."""
    if w == NWAVES - 1 and NGROUPS % WAVE:
        return "E" * (NGROUPS % WAVE)
    return "EDD" if w % 3 == 2 else "EED"


def _group_form(g):
    w, k = divmod(g, WAVE)
    return _wave_pattern(w)[k]


def _col_band(g, r):
    """PE column band for tile r of group g (rotated for E, diagonal for D)."""
    return (r + g) % 4 if _group_form(g) == "E" else r


_nc_cache = {}


def _build_nc():
    import concourse.mybir as mybir
    import concourse.tile as tile
    from concourse import bacc

    nc = bacc.Bacc()
    pt_d = nc.dram_tensor(
        "pt", [PARTS, NGROUPS * NB], mybir.dt.float16, kind="ExternalInput"
    )
    w_d = nc.dram_tensor(
        "wblk", [PARTS, NGROUPS * 32], mybir.dt.float16, kind="ExternalInput"
    )
    lam_d = nc.dram_tensor(
        "lamt", [PARTS, NGROUPS], mybir.dt.float16, kind="ExternalInput"
    )
    out_d = nc.dram_tensor("out", [4, NB], mybir.dt.float32, kind="ExternalOutput")

    with tile.TileContext(nc) as tc:
        with (
            tc.tile_pool(name="const", bufs=1) as cpool,
            tc.tile_pool(name="wtp", bufs=len(WT_CHUNKS)) as wpool,
            tc.tile_pool(name="ptp", bufs=len(PT_CHUNKS)) as ppool,
            tc.tile_pool(name="qp", bufs=3) as qpool,
            tc.tile_pool(name="psp", bufs=2, space="PSUM") as pspool,
            tc.tile_pool(name="accp", bufs=1, space="PSUM") as apool,
        ):
            lam_sb = cpool.tile([PARTS, NGROUPS], mybir.dt.float16)
            acc = apool.tile([PARTS, NB], mybir.dt.float32)

            # ---- stage all input DMAs (SBUF holds everything) ----
            wtiles = []  # (tile, first_group, ngroups)
            g0 = 0
            for ci, ch in enumerate(WT_CHUNKS):
                wt = wpool.tile([PARTS, max(WT_CHUNKS) * 32], mybir.dt.float16,
                                tag="wt")
                nc.sync.dma_start(
                    out=wt[:, : ch * 32],
                    in_=w_d[:, g0 * 32 : (g0 + ch) * 32],
                )
                wtiles.append((wt, g0, ch))
                if ci == 0:
                    nc.sync.dma_start(out=lam_sb[:, :], in_=lam_d[:, :])
                g0 += ch

            ptiles = []
            group_pt = {}  # group -> (tile_idx, offset_in_chunk)
            g0 = 0
            for ci, ch in enumerate(PT_CHUNKS):
                pt = ppool.tile([PARTS, max(PT_CHUNKS) * NB], mybir.dt.float16,
                                tag="pt")
                nc.sync.dma_start(
                    out=pt[:, : ch * NB],
                    in_=pt_d[:, g0 * NB : (g0 + ch) * NB],
                )
                ptiles.append(pt)
                for k in range(ch):
                    group_pt[g0 + k] = (ci, k)
                g0 += ch

            def wt_slice(g, r):
                for wt, wg0, wch in wtiles:
                    if wg0 <= g < wg0 + wch:
                        c0 = (g - wg0) * 32
                        return wt[32 * r : 32 * r + 32, c0 : c0 + 32]
                raise AssertionError(g)

            # ---- pipeline over waves ----
            q_slices = {}  # group -> (q_tile, slot)
            n_rounds = -(-NGROUPS // 4)
            rounds_emitted = 0
            last_round_of_pos = {}
            for r in range(n_rounds):
                for j in range(4):
                    if r * 4 + j < NGROUPS:
                        last_round_of_pos[j] = r

            def emit_mm2_rounds(limit_group):
                nonlocal rounds_emitted
                while rounds_emitted < n_rounds:
                    r = rounds_emitted
                    hi = min(r * 4 + 4, NGROUPS)
                    if hi > limit_group:
                        return
                    for j in range(4):
                        g = r * 4 + j
                        if g >= NGROUPS:
                            break
                        qt, slot = q_slices.pop(g)
                        nc.tensor.matmul(
                            acc[32 * j : 32 * j + 1, :],
                            lam_sb[:, g : g + 1],
                            qt[:, slot * NB : (slot + 1) * NB],
                            start=(r == 0),
                            stop=(r == last_round_of_pos[j]),
                            tile_position=(0, 32 * j),
                        )
                    rounds_emitted += 1

            for w in range(NWAVES):
                pat = _wave_pattern(w)
                ng = len(pat)
                wg0 = w * WAVE
                psT = pspool.tile([PARTS, WAVE * NB], mybir.dt.float32, tag="psT")
                for k in range(ng):
                    g = wg0 + k
                    ci, off = group_pt[g]
                    for r in range(4):
                        c = _col_band(g, r)
                        nc.tensor.matmul(
                            psT[32 * c : 32 * c + 32, k * NB : (k + 1) * NB],
                            wt_slice(g, r),
                            ptiles[ci][32 * r : 32 * r + 32,
                                       off * NB : (off + 1) * NB],
                            start=True,
                            stop=True,
                            tile_position=(32 * r, 32 * c),
                        )
                # drain PSUM: ACT squares the eigen span, DVE fuses the
                # direct span with pt (one op, no copy)
                q = qpool.tile([PARTS, WAVE * NB], mybir.dt.float16, tag="q")
                n_e = pat.count("E")
                if n_e:
                    nc.scalar.square(q[:, : n_e * NB], psT[:, : n_e * NB])
                k = n_e
                while k < ng:  # direct span, split at pt-chunk boundaries
                    ci, off = group_pt[wg0 + k]
                    k1 = k + 1
                    while k1 < ng and group_pt[wg0 + k1] == (ci, off + k1 - k):
                        k1 += 1
                    nc.vector.tensor_mul(
                        q[:, k * NB : k1 * NB],
                        psT[:, k * NB : k1 * NB],
                        ptiles[ci][:, off * NB : (off + k1 - k) * NB],
                    )
                    k = k1
                for k in range(ng):
                    q_slices[wg0 + k] = (q, k)
                emit_mm2_rounds(wg0)
            emit_mm2_rounds(NGROUPS)

            # evacuate the 4 accumulator rows (partition-aligned copies)
            out_sb = cpool.tile([PARTS, NB], mybir.dt.float32)
            for j in range(4):
                if j % 2:
                    nc.scalar.copy(
                        out_sb[32 * j : 32 * j + 1, :], acc[32 * j : 32 * j + 1, :]
                    )
                else:
                    nc.vector.tensor_copy(
                        out_sb[32 * j : 32 * j + 1, :], acc[32 * j : 32 * j + 1, :]
                    )
                nc.sync.dma_start(
                    out=out_d[j : j + 1, :], in_=out_sb[32 * j : 32 * j + 1, :]
                )
    if not nc.is_finalized():
        nc.finalize()
    return nc


def _get_nc():
    if "nc" not in _nc_cache:
        _nc_cache["nc"] = _build_nc()
    return _nc_cache["nc"]


def _host_prep_weights(integral_coeffs):
    """coeffs [G,G,C] -> (wblk [128, NGROUPS*32] fp16, lamt [128, NGROUPS] fp16).

    Per group g, tile r: wblk[32r + 8q + i, 32g + 8q + k] holds, for cell
    ct = 16g + 4r + q, either V_ct[i,k] (eigen groups) or S_ct[i,k]
    (direct groups).  lamt[32c + 8q + k, g] holds the matching psum-
    partition weight for mm2: lam_ct[k] with r=(c-g)%4 for eigen groups,
    1.0 for direct groups.
    """
    ii, jj = np.triu_indices(P)
    wq = integral_coeffs.reshape(CELLS, len(ii)).astype(np.float64)
    S = np.zeros((CELLS, P, P), np.float64)
    np.add.at(S, (slice(None), ii, jj), 0.5 * wq)
    np.add.at(S, (slice(None), jj, ii), 0.5 * wq)
    lam, V = np.linalg.eigh(S)

    S_p = np.zeros((CELLS_PAD, P, P))
    S_p[:CELLS] = S
    lam_p = np.zeros((CELLS_PAD, P))
    lam_p[:CELLS] = lam
    V_p = np.zeros((CELLS_PAD, P, P))
    V_p[:CELLS] = V

    wblk = np.zeros((PARTS, NGROUPS * 32), np.float32)
    lamt = np.zeros((PARTS, NGROUPS), np.float32)
    for g in range(NGROUPS):
        eigen = _group_form(g) == "E"
        M = V_p if eigen else S_p
        for r in range(4):
            for q in range(4):
                ct = 16 * g + 4 * r + q
                wblk[32 * r + 8 * q : 32 * r + 8 * q + 8,
                     32 * g + 8 * q : 32 * g + 8 * q + 8] = M[ct]
        if eigen:
            for c in range(4):
                r = (c - g) % 4
                for q in range(4):
                    ct = 16 * g + 4 * r + q
                    lamt[32 * c + 8 * q : 32 * c + 8 * q + 8, g] = lam_p[ct]
        else:
            lamt[:, g] = 1.0
    return (
        np.ascontiguousarray(wblk.astype(np.float16)),
        np.ascontiguousarray(lamt.astype(np.float16)),
    )


def _host_prep_param(param_tensor):
    """param [B,G,G,P] f32 -> list of per-core [128, NGROUPS*NB] fp16 arrays."""
    flat = param_tensor.reshape(B, CELLS * P)
    out = []
    for c in range(NCORES):
        shard = flat[c * NB : (c + 1) * NB]
        pad = np.zeros((NB, CELLS_PAD * P), np.float32)
        pad[:, : CELLS * P] = shard
        # (b, g, p) -> (p, g, b)
        pt = (
            pad.reshape(NB, NGROUPS, PARTS)
            .transpose(2, 1, 0)
            .reshape(PARTS, NGROUPS * NB)
            .astype(np.float16)
        )
        out.append(np.ascontiguousarray(pt))
    return out


def _run(param_tensor, integral_coeffs, trace=False, **run_kwargs):
    from concourse.bass_utils import run_bass_kernel_spmd

    nc = _get_nc()
    wblk, lamt = _host_prep_weights(np.asarray(integral_coeffs, np.float32))
    pts = _host_prep_param(np.asarray(param_tensor, np.float32))
    in_maps = [{"pt": pts[c], "wblk": wblk, "lamt": lamt} for c in range(NCORES)]
    res = run_bass_kernel_spmd(
        nc, in_maps, core_ids=list(range(NCORES)), trace=trace, **run_kwargs
    )
    out = np.concatenate(
        [res.results[c]["out"].sum(axis=0).reshape(NB) for c in range(NCORES)]
    ).astype(np.float32)
    return out, res


def kernel(param_tensor, integral_coeffs):
    out, _ = _run(param_tensor, integral_coeffs)
    return out
